# revision 17
# baseline (speedup 1.0000x reference)
"""Trainium2 Bass kernel for nn_Attention (32-head attention, partial rotary,
dense softmax) sharded 4-heads-per-core across 8 NeuronCores.

Self-contained: takes full unsharded inputs, returns the full output.

Design notes (per core, heads h = 4c..4c+3, N=2048 tokens, d_head=256, e=128):
  - All projections computed in transposed [feature, token] layout so no
    on-device transposes are needed anywhere.
  - rotate_half is pre-baked on the host into 64 extra "swapped" weight
    columns per q/k head, so rotary is 3 aligned elementwise ops.
  - V is projected through this head's slice of Wproj *before* the
    attention-weighted sum (U_h = Wproj_h @ v_h), shrinking the big
    j-contraction from d=256 to e=128 and making the output projection free.
  - Softmax without max-subtraction (|score*scale| < ~1 by construction);
    denominators via ones-vector matmuls; normalization + bias + head/core
    reduction on the host (cheap: 8MB of partial outputs).
  - bf16 matmul inputs, fp32 PSUM accumulation.
"""

import sys

sys.path.insert(0, "/opt/trn_rl_repo")

import numpy as np
import ml_dtypes

import concourse.bacc as bacc
import concourse.tile as tile
from concourse import mybir
from concourse.bass_utils import run_bass_kernel_spmd

DIM = 128
HEADS = 32
DH = 256          # per-head dim
ROT = 64          # partial rotary width
N = 2048
NCORES = 8
HPC = HEADS // NCORES  # heads per core = 4
SCALE = float(DIM) ** -0.5

BF16 = mybir.dt.bfloat16
FP8 = mybir.dt.float8e4
F32 = mybir.dt.float32
EXP = mybir.ActivationFunctionType.Exp
DR = mybir.MatmulPerfMode.DoubleRow

BF16_NP = ml_dtypes.bfloat16
FP8_NP = ml_dtypes.float8_e4m3


def build_nc(n=N):
    """Build the per-core Bass program (identical for all cores; SPMD)."""
    assert n % 512 == 0
    nch = n // 512   # 512-wide query chunks
    njt = n // 128   # 128-wide key tiles

    nc = bacc.Bacc("TRN2", target_bir_lowering=False, debug=False,
                   num_devices=NCORES)

    xT = nc.dram_tensor("xT", [128, n], BF16, kind="ExternalInput")
    wq = nc.dram_tensor("wq", [128, HPC, DH + ROT], BF16, kind="ExternalInput")
    wk = nc.dram_tensor("wk", [128, HPC, DH + ROT], BF16, kind="ExternalInput")
    wv = nc.dram_tensor("wv", [128, HPC, DH], BF16, kind="ExternalInput")
    wp = nc.dram_tensor("wp", [128, HPC, 2, 128], BF16, kind="ExternalInput")
    cosT = nc.dram_tensor("cosT", [ROT, n], F32, kind="ExternalInput")
    sinT = nc.dram_tensor("sinT", [ROT, n], F32, kind="ExternalInput")
    uv = nc.dram_tensor("uv", [HPC, 128, n], F32, kind="ExternalOutput")
    ssum = nc.dram_tensor("ssum", [HPC, 128, n], F32, kind="ExternalOutput")

    with tile.TileContext(nc) as tc:
        with (
            tc.tile_pool(name="consts", bufs=1) as consts,
            tc.tile_pool(name="hd", bufs=2) as hd,
            tc.tile_pool(name="es", bufs=3) as es,
            tc.tile_pool(name="tmp", bufs=4) as tmp,
            tc.tile_pool(name="ps", bufs=2, space="PSUM") as ps,
        ):
            wq_sb = consts.tile([128, HPC, DH + ROT], BF16)
            nc.sync.dma_start(out=wq_sb, in_=wq[:, :, :])
            xT_sb = consts.tile([128, n], BF16)
            for ci in range(nch):   # chunked so the first matmul starts early
                sl = slice(ci * 512, ci * 512 + 512)
                nc.sync.dma_start(out=xT_sb[:, sl], in_=xT[:, sl])
            wk_sb = consts.tile([128, HPC, DH + ROT], BF16)
            nc.gpsimd.dma_start(out=wk_sb, in_=wk[:, :, :])
            wv_sb = consts.tile([128, HPC, DH], BF16)
            nc.gpsimd.dma_start(out=wv_sb, in_=wv[:, :, :])
            wp_sb = consts.tile([128, HPC, 2, 128], BF16)
            nc.gpsimd.dma_start(out=wp_sb, in_=wp[:, :, :, :])
            cos_sb = consts.tile([ROT, n], F32)
            nc.gpsimd.dma_start(out=cos_sb, in_=cosT[:, :])
            sin_sb = consts.tile([ROT, n], F32)
            nc.gpsimd.dma_start(out=sin_sb, in_=sinT[:, :])

            for h in range(HPC):
                qT_sb = hd.tile([128, 2, n], FP8, tag="q")
                kT_sb = hd.tile([128, 2, n], FP8, tag="k")
                vT_sb = hd.tile([128, 2, n], BF16, tag="v")
                U_sb = hd.tile([128, njt, 128], BF16, tag="u")

                # ---- q/k projections with fused rotary ----
                for w_sb, outT in ((wq_sb, qT_sb), (wk_sb, kT_sb)):
                    for dt in range(2):
                        dsl = slice(dt * 128, dt * 128 + 128)
                        for ci in range(nch):
                            sl = slice(ci * 512, ci * 512 + 512)
                            psq = ps.tile([128, 512], F32, tag="pqk", bufs=2)
                            nc.tensor.matmul(psq, w_sb[:, h, dsl],
                                             xT_sb[:, sl], start=True, stop=True)
                            if dt == 0:
                                # swapped-partner projection for rotate_half
                                psqs = ps.tile([64, 512], F32, tag="pqk", bufs=2)
                                nc.tensor.matmul(psqs, w_sb[:, h, DH:DH + ROT],
                                                 xT_sb[:, sl], start=True, stop=True)
                                t1 = tmp.tile([64, 512], F32, tag="t1")
                                t2 = tmp.tile([64, 512], F32, tag="t2")
                                nc.vector.tensor_mul(t1, psq[0:ROT, :], cos_sb[:, sl])
                                nc.vector.tensor_mul(t2, psqs[:, :], sin_sb[:, sl])
                                nc.vector.tensor_add(outT[0:ROT, dt, sl], t1, t2)
                                nc.scalar.copy(outT[ROT:128, dt, sl], psq[ROT:128, :])
                            else:
                                nc.scalar.copy(outT[:, dt, sl], psq)

                # ---- v projection (transposed layout) ----
                for dt in range(2):
                    dsl = slice(dt * 128, dt * 128 + 128)
                    for ci in range(nch):
                        sl = slice(ci * 512, ci * 512 + 512)
                        psv = ps.tile([128, 512], F32, tag="pqk", bufs=2)
                        nc.tensor.matmul(psv, wv_sb[:, h, dsl], xT_sb[:, sl],
                                         start=True, stop=True)
                        nc.scalar.copy(vT_sb[:, dt, sl], psv)

                # ---- U_h[j, e] = sum_d vT[d, j] * WprojT[(h,d), e] ----
                for jt in range(njt):
                    jsl = slice(jt * 128, jt * 128 + 128)
                    psu = ps.tile([128, 128], F32, tag="pqk", bufs=2)
                    for dt in range(2):
                        nc.tensor.matmul(psu, vT_sb[:, dt, jsl], wp_sb[:, h, dt, :],
                                         start=(dt == 0), stop=(dt == 1))
                    nc.vector.tensor_copy(U_sb[:, jt, :], psu)

                # ---- attention: fp8-DR scores -> paired exp -> bf16 U-sum ----
                for ci in range(nch):
                    isl = slice(ci * 512, ci * 512 + 512)
                    expS = es.tile([128, njt, 512], BF16, tag="e")
                    psuv = ps.tile([128, 512], F32, tag="puv", bufs=2)
                    for jp in range(njt // 2):
                        pss = ps.tile([128, 1024], F32, tag="ps", bufs=2)
                        for u in range(2):
                            jt = 2 * jp + u
                            jsl = slice(jt * 128, jt * 128 + 128)
                            nc.tensor.matmul(pss[:, u * 512:(u + 1) * 512],
                                             kT_sb[:, :, jsl], qT_sb[:, :, isl],
                                             start=True, stop=True, perf_mode=DR)
                        nc.scalar.activation(
                            expS[:, 2 * jp:2 * jp + 2, :].rearrange("p j i -> p (j i)"),
                            pss, EXP, scale=SCALE)
                        for u in range(2):
                            jt = 2 * jp + u
                            nc.tensor.matmul(psuv, U_sb[:, jt, :], expS[:, jt, :],
                                             start=(jt == 0), stop=(jt == njt - 1))
                    # denominator partials (sum over key tiles), off the PE:
                    # DVE reduces 6 tiles, GpSimd add-tree the other 10.
                    nd = max(2, njt * 6 // 16)
                    sdve = tmp.tile([128, 512], F32, tag="sdve", bufs=2)
                    nc.vector.tensor_reduce(
                        sdve, expS[:, 0:nd, :].rearrange("p j i -> p i j"),
                        axis=mybir.AxisListType.X, op=mybir.AluOpType.add)
                    g = [expS[:, nd + t, :] for t in range(njt - nd)]
                    while len(g) > 2:
                        nxt = []
                        for a, b in zip(g[0::2], g[1::2]):
                            gt = tmp.tile([128, 512], BF16, tag="gtree", bufs=8)
                            nc.gpsimd.tensor_add(gt, a, b)
                            nxt.append(gt)
                        if len(g) % 2:
                            nxt.append(g[-1])
                        g = nxt
                    gsum = tmp.tile([128, 512], F32, tag="gsum", bufs=2)
                    nc.gpsimd.tensor_add(gsum, g[0], g[1])
                    osm = tmp.tile([128, 512], F32, tag="osm", bufs=2)
                    nc.vector.tensor_add(osm, sdve, gsum)
                    ouv = tmp.tile([128, 512], F32, tag="ouv", bufs=2)
                    nc.vector.tensor_copy(ouv, psuv)
                    nc.sync.dma_start(out=uv[h, :, isl], in_=ouv)
                    nc.sync.dma_start(out=ssum[h, :, isl], in_=osm)

    nc.compile()
    return nc


_PERM = np.concatenate([np.arange(32, 64), np.arange(0, 32)])


def prep_core(core, x, Wqkv, Wproj, rot, n=N):
    """Build the per-core input map (numpy, host-side sharding/layout)."""
    hs = slice(core * HPC, (core + 1) * HPC)
    W4 = Wqkv.reshape(3, HEADS, DH, DIM)

    def qk_aug(w):  # w: [HPC, DH, DIM] -> [128, HPC, DH+ROT]
        wt = np.ascontiguousarray(w.transpose(2, 0, 1))          # [128, HPC, 256]
        sw = np.ascontiguousarray(w[:, _PERM, :].transpose(2, 0, 1))  # [128, HPC, 64]
        return np.concatenate([wt, sw], axis=2).astype(BF16_NP)

    wq_h = qk_aug(W4[0, hs])
    wk_h = qk_aug(W4[1, hs])
    wv_h = np.ascontiguousarray(W4[2, hs].transpose(2, 0, 1)).astype(BF16_NP)
    # Wproj [128 e, INNER] -> this core's [(h,d)] rows as [128 d_in, HPC, 2, 128 e]
    Wp4 = Wproj.reshape(DIM, HEADS, DH)[:, hs, :]                # [128 e, HPC, 256]
    wp_h = np.ascontiguousarray(
        Wp4.reshape(DIM, HPC, 2, 128).transpose(3, 1, 2, 0)).astype(BF16_NP)

    cosT = np.ascontiguousarray(np.cos(rot).T).astype(np.float32)   # [64, n]
    sinT = np.ascontiguousarray(np.sin(rot).T).astype(np.float32)
    sinT[:32] *= -1.0   # sign of rotate_half for output rows 0:32

    xT = np.ascontiguousarray(x.reshape(n, DIM).T).astype(BF16_NP)  # [128, n]

    return {
        "xT": xT, "wq": wq_h, "wk": wk_h, "wv": wv_h, "wp": wp_h,
        "cosT": cosT, "sinT": sinT,
    }


def postprocess(results, bproj, n=N):
    """Normalize per (head, query), sum heads/cores, add bias -> [1, n, 128]."""
    acc = np.zeros((DIM, n), np.float64)
    for r in results:
        uv = np.asarray(r["uv"], np.float64)       # [HPC, 128, n]
        ssum = np.asarray(r["ssum"], np.float64)   # [HPC, 128, n] partials
        ssum = ssum.sum(axis=1)                    # [HPC, n]
        acc += (uv / ssum[:, None, :]).sum(axis=0)
    out = acc.T + np.asarray(bproj, np.float64)[None, :]
    return out.astype(np.float32).reshape(1, n, DIM)


_NC_CACHE = {}


def _get_nc(n=N):
    if n not in _NC_CACHE:
        _NC_CACHE[n] = build_nc(n)
    return _NC_CACHE[n]


def kernel(x, Wqkv, Wproj, bproj, rotary_pos_emb):
    x = np.asarray(x, np.float32)
    Wqkv = np.asarray(Wqkv, np.float32)
    Wproj = np.asarray(Wproj, np.float32)
    bproj = np.asarray(bproj, np.float32)
    rot = np.asarray(rotary_pos_emb, np.float32)

    nc = _get_nc(N)
    in_maps = [prep_core(c, x, Wqkv, Wproj, rot, N) for c in range(NCORES)]
    res = run_bass_kernel_spmd(nc, in_maps, core_ids=list(range(NCORES)))
    return postprocess(res.results, bproj, N)


# revision 22
# speedup vs baseline: 1.0658x; 1.0658x over previous
"""Trainium2 Bass kernel for nn_Attention (32-head attention, partial rotary,
dense softmax) sharded 4-heads-per-core across 8 NeuronCores.

Self-contained: takes full unsharded inputs, returns the full output.

Design notes (per core, heads h = 4c..4c+3, N=2048 tokens, d_head=256, e=128):
  - All projections computed in transposed [feature, token] layout so no
    on-device transposes are needed anywhere.
  - rotate_half is pre-baked on the host into 64 extra "swapped" weight
    columns per q/k head, so rotary is 3 aligned elementwise ops.
  - V is projected through this head's slice of Wproj *before* the
    attention-weighted sum (U_h = Wproj_h @ v_h), shrinking the big
    j-contraction from d=256 to e=128 and making the output projection free.
  - Softmax without max-subtraction (|score*scale| < ~1 by construction);
    denominators via ones-vector matmuls; normalization + bias + head/core
    reduction on the host (cheap: 8MB of partial outputs).
  - bf16 matmul inputs, fp32 PSUM accumulation.
"""

import sys

sys.path.insert(0, "/opt/trn_rl_repo")

import numpy as np
import ml_dtypes

import concourse.bacc as bacc
import concourse.tile as tile
from concourse import mybir
from concourse.bass_utils import run_bass_kernel_spmd

DIM = 128
HEADS = 32
DH = 256          # per-head dim
ROT = 64          # partial rotary width
N = 2048
NCORES = 8
HPC = HEADS // NCORES  # heads per core = 4
SCALE = float(DIM) ** -0.5

BF16 = mybir.dt.bfloat16
FP8 = mybir.dt.float8e4
F32 = mybir.dt.float32
EXP = mybir.ActivationFunctionType.Exp
DR = mybir.MatmulPerfMode.DoubleRow

BF16_NP = ml_dtypes.bfloat16
FP8_NP = ml_dtypes.float8_e4m3


def build_nc(n=N):
    """Build the per-core Bass program (identical for all cores; SPMD)."""
    assert n % 512 == 0
    nch = n // 512   # 512-wide query chunks
    njt = n // 128   # 128-wide key tiles

    nc = bacc.Bacc("TRN2", target_bir_lowering=False, debug=False,
                   num_devices=NCORES)

    xT = nc.dram_tensor("xT", [128, n], BF16, kind="ExternalInput")
    wq = nc.dram_tensor("wq", [128, HPC, DH + ROT], BF16, kind="ExternalInput")
    wk = nc.dram_tensor("wk", [128, HPC, DH + ROT], BF16, kind="ExternalInput")
    wv = nc.dram_tensor("wv", [128, HPC, DH], BF16, kind="ExternalInput")
    wp = nc.dram_tensor("wp", [128, HPC, 2, 128], BF16, kind="ExternalInput")
    cosT = nc.dram_tensor("cosT", [ROT, n], F32, kind="ExternalInput")
    sinT = nc.dram_tensor("sinT", [ROT, n], F32, kind="ExternalInput")
    uv = nc.dram_tensor("uv", [HPC, 128, n], F32, kind="ExternalOutput")
    ssum = nc.dram_tensor("ssum", [HPC, 128, n], F32, kind="ExternalOutput")

    with tile.TileContext(nc) as tc:
        with (
            tc.tile_pool(name="consts", bufs=1) as consts,
            tc.tile_pool(name="hd", bufs=2) as hd,
            tc.tile_pool(name="es", bufs=3) as es,
            tc.tile_pool(name="tmp", bufs=4) as tmp,
            tc.tile_pool(name="ps", bufs=2, space="PSUM") as ps,
        ):
            wq_sb = consts.tile([128, HPC, DH + ROT], BF16)
            nc.sync.dma_start(out=wq_sb, in_=wq[:, :, :])
            xT_sb = consts.tile([128, n], BF16)
            for ci in range(nch):   # chunked so the first matmul starts early
                sl = slice(ci * 512, ci * 512 + 512)
                nc.sync.dma_start(out=xT_sb[:, sl], in_=xT[:, sl])
            wk_sb = consts.tile([128, HPC, DH + ROT], BF16)
            nc.gpsimd.dma_start(out=wk_sb, in_=wk[:, :, :])
            wv_sb = consts.tile([128, HPC, DH], BF16)
            nc.gpsimd.dma_start(out=wv_sb, in_=wv[:, :, :])
            wp_sb = consts.tile([128, HPC, 2, 128], BF16)
            nc.gpsimd.dma_start(out=wp_sb, in_=wp[:, :, :, :])
            cos_sb = consts.tile([ROT, n], F32)
            nc.gpsimd.dma_start(out=cos_sb, in_=cosT[:, :])
            sin_sb = consts.tile([ROT, n], F32)
            nc.gpsimd.dma_start(out=sin_sb, in_=sinT[:, :])

            def alloc_head_tiles():
                qT_sb = hd.tile([128, 2, n], FP8, tag="q", name="qT_sb")
                kT_sb = hd.tile([128, 2, n], FP8, tag="k", name="kT_sb")
                vT_sb = hd.tile([128, 2, n], BF16, tag="v", name="vT_sb")
                U_sb = hd.tile([128, njt, 128], BF16, tag="u", name="U_sb")
                return qT_sb, kT_sb, vT_sb, U_sb

            def emit_qk(h, w_sb, outT):
                # q or k projection with fused rotary
                for dt in range(2):
                    dsl = slice(dt * 128, dt * 128 + 128)
                    for ci in range(nch):
                        sl = slice(ci * 512, ci * 512 + 512)
                        psq = ps.tile([128, 512], F32, tag="pqk", bufs=3,
                                      name="psq")
                        nc.tensor.matmul(psq, w_sb[:, h, dsl],
                                         xT_sb[:, sl], start=True, stop=True)
                        if dt == 0:
                            # swapped-partner projection for rotate_half
                            psqs = ps.tile([64, 512], F32, tag="pqk", bufs=3,
                                           name="psqs")
                            nc.tensor.matmul(psqs, w_sb[:, h, DH:DH + ROT],
                                             xT_sb[:, sl], start=True, stop=True)
                            t1 = tmp.tile([64, 512], F32, tag="t1", name="t1")
                            t2 = tmp.tile([64, 512], F32, tag="t2", name="t2")
                            nc.vector.tensor_mul(t1, psq[0:ROT, :], cos_sb[:, sl])
                            nc.vector.tensor_mul(t2, psqs[:, :], sin_sb[:, sl])
                            nc.vector.tensor_add(outT[0:ROT, dt, sl], t1, t2)
                            nc.scalar.copy(outT[ROT:128, dt, sl], psq[ROT:128, :])
                        else:
                            nc.scalar.copy(outT[:, dt, sl], psq)

            def emit_v(h, vT_sb):
                for dt in range(2):
                    dsl = slice(dt * 128, dt * 128 + 128)
                    for ci in range(nch):
                        sl = slice(ci * 512, ci * 512 + 512)
                        psv = ps.tile([128, 512], F32, tag="pqk", bufs=3,
                                      name="psv")
                        nc.tensor.matmul(psv, wv_sb[:, h, dsl], xT_sb[:, sl],
                                         start=True, stop=True)
                        nc.scalar.copy(vT_sb[:, dt, sl], psv)

            def emit_u(h, vT_sb, U_sb):
                # U_h[j, e] = sum_d vT[d, j] * WprojT[(h,d), e]
                for jt in range(njt):
                    jsl = slice(jt * 128, jt * 128 + 128)
                    psu = ps.tile([128, 128], F32, tag="pqk", bufs=3, name="psu")
                    for dt in range(2):
                        nc.tensor.matmul(psu, vT_sb[:, dt, jsl], wp_sb[:, h, dt, :],
                                         start=(dt == 0), stop=(dt == 1))
                    nc.vector.tensor_copy(U_sb[:, jt, :], psu)

            def build_parts(h, tiles):
                qT_sb, kT_sb, vT_sb, U_sb = tiles
                return [
                    lambda: emit_qk(h, wq_sb, qT_sb),
                    lambda: emit_qk(h, wk_sb, kT_sb),
                    lambda: emit_v(h, vT_sb),
                    lambda: emit_u(h, vT_sb, U_sb),
                ]

            cur = alloc_head_tiles()
            for part in build_parts(0, cur):
                part()

            for h in range(HPC):
                qT_sb, kT_sb, vT_sb, U_sb = cur
                if h + 1 < HPC:
                    nxt = alloc_head_tiles()
                    nxt_parts = build_parts(h + 1, nxt)
                else:
                    nxt, nxt_parts = None, None

                # ---- attention: fp8-DR scores -> paired exp -> bf16 U-sum ----
                # next head's build work is interleaved between query chunks
                # so the PE fills attention stalls and DVE/ACT load spreads.
                for ci in range(nch):
                    isl = slice(ci * 512, ci * 512 + 512)
                    expS = es.tile([128, njt, 512], BF16, tag="e")
                    psuv = ps.tile([128, 512], F32, tag="puv", bufs=1)
                    for jp in range(njt // 2):
                        pss = ps.tile([128, 1024], F32, tag="ps", bufs=2)
                        for u in range(2):
                            jt = 2 * jp + u
                            jsl = slice(jt * 128, jt * 128 + 128)
                            nc.tensor.matmul(pss[:, u * 512:(u + 1) * 512],
                                             kT_sb[:, :, jsl], qT_sb[:, :, isl],
                                             start=True, stop=True, perf_mode=DR)
                        nc.scalar.activation(
                            expS[:, 2 * jp:2 * jp + 2, :].rearrange("p j i -> p (j i)"),
                            pss, EXP, scale=SCALE)
                        for u in range(2):
                            jt = 2 * jp + u
                            nc.tensor.matmul(psuv, U_sb[:, jt, :], expS[:, jt, :],
                                             start=(jt == 0), stop=(jt == njt - 1))
                    # denominator partials (sum over key tiles), off the PE:
                    # DVE reduces 6 tiles, GpSimd add-tree the other 10.
                    nd = max(2, njt * 6 // 16)
                    sdve = tmp.tile([128, 512], F32, tag="sdve", bufs=2)
                    nc.vector.tensor_reduce(
                        sdve, expS[:, 0:nd, :].rearrange("p j i -> p i j"),
                        axis=mybir.AxisListType.X, op=mybir.AluOpType.add)
                    g = [expS[:, nd + t, :] for t in range(njt - nd)]
                    while len(g) > 2:
                        lvl = []
                        for a, b in zip(g[0::2], g[1::2]):
                            gt = tmp.tile([128, 512], BF16, tag="gtree", bufs=8)
                            nc.gpsimd.tensor_add(gt, a, b)
                            lvl.append(gt)
                        if len(g) % 2:
                            lvl.append(g[-1])
                        g = lvl
                    gsum = tmp.tile([128, 512], F32, tag="gsum", bufs=2)
                    nc.gpsimd.tensor_add(gsum, g[0], g[1])
                    osm = tmp.tile([128, 512], F32, tag="osm", bufs=2)
                    nc.vector.tensor_add(osm, sdve, gsum)
                    ouv = tmp.tile([128, 512], F32, tag="ouv", bufs=2)
                    nc.vector.tensor_copy(ouv, psuv)
                    nc.sync.dma_start(out=uv[h, :, isl], in_=ouv)
                    nc.sync.dma_start(out=ssum[h, :, isl], in_=osm)
                    if nxt_parts is not None and ci < len(nxt_parts):
                        nxt_parts[ci]()
                if nxt_parts is not None:
                    for pi in range(nch, len(nxt_parts)):
                        nxt_parts[pi]()
                cur = nxt

    nc.compile()
    return nc


_PERM = np.concatenate([np.arange(32, 64), np.arange(0, 32)])


def prep_core(core, x, Wqkv, Wproj, rot, n=N):
    """Build the per-core input map (numpy, host-side sharding/layout)."""
    hs = slice(core * HPC, (core + 1) * HPC)
    W4 = Wqkv.reshape(3, HEADS, DH, DIM)

    def qk_aug(w):  # w: [HPC, DH, DIM] -> [128, HPC, DH+ROT]
        wt = np.ascontiguousarray(w.transpose(2, 0, 1))          # [128, HPC, 256]
        sw = np.ascontiguousarray(w[:, _PERM, :].transpose(2, 0, 1))  # [128, HPC, 64]
        return np.concatenate([wt, sw], axis=2).astype(BF16_NP)

    wq_h = qk_aug(W4[0, hs])
    wk_h = qk_aug(W4[1, hs])
    wv_h = np.ascontiguousarray(W4[2, hs].transpose(2, 0, 1)).astype(BF16_NP)
    # Wproj [128 e, INNER] -> this core's [(h,d)] rows as [128 d_in, HPC, 2, 128 e]
    Wp4 = Wproj.reshape(DIM, HEADS, DH)[:, hs, :]                # [128 e, HPC, 256]
    wp_h = np.ascontiguousarray(
        Wp4.reshape(DIM, HPC, 2, 128).transpose(3, 1, 2, 0)).astype(BF16_NP)

    cosT = np.ascontiguousarray(np.cos(rot).T).astype(np.float32)   # [64, n]
    sinT = np.ascontiguousarray(np.sin(rot).T).astype(np.float32)
    sinT[:32] *= -1.0   # sign of rotate_half for output rows 0:32

    xT = np.ascontiguousarray(x.reshape(n, DIM).T).astype(BF16_NP)  # [128, n]

    return {
        "xT": xT, "wq": wq_h, "wk": wk_h, "wv": wv_h, "wp": wp_h,
        "cosT": cosT, "sinT": sinT,
    }


def postprocess(results, bproj, n=N):
    """Normalize per (head, query), sum heads/cores, add bias -> [1, n, 128]."""
    acc = np.zeros((DIM, n), np.float64)
    for r in results:
        uv = np.asarray(r["uv"], np.float64)       # [HPC, 128, n]
        ssum = np.asarray(r["ssum"], np.float64)   # [HPC, 128, n] partials
        ssum = ssum.sum(axis=1)                    # [HPC, n]
        acc += (uv / ssum[:, None, :]).sum(axis=0)
    out = acc.T + np.asarray(bproj, np.float64)[None, :]
    return out.astype(np.float32).reshape(1, n, DIM)


_NC_CACHE = {}


def _get_nc(n=N):
    if n not in _NC_CACHE:
        _NC_CACHE[n] = build_nc(n)
    return _NC_CACHE[n]


def kernel(x, Wqkv, Wproj, bproj, rotary_pos_emb):
    x = np.asarray(x, np.float32)
    Wqkv = np.asarray(Wqkv, np.float32)
    Wproj = np.asarray(Wproj, np.float32)
    bproj = np.asarray(bproj, np.float32)
    rot = np.asarray(rotary_pos_emb, np.float32)

    nc = _get_nc(N)
    in_maps = [prep_core(c, x, Wqkv, Wproj, rot, N) for c in range(NCORES)]
    res = run_bass_kernel_spmd(nc, in_maps, core_ids=list(range(NCORES)))
    return postprocess(res.results, bproj, N)


# revision 23
# speedup vs baseline: 1.0759x; 1.0095x over previous
"""Trainium2 Bass kernel for nn_Attention (32-head attention, partial rotary,
dense softmax) sharded 4-heads-per-core across 8 NeuronCores.

Self-contained: takes full unsharded inputs, returns the full output.

Design notes (per core, heads h = 4c..4c+3, N=2048 tokens, d_head=256, e=128):
  - All projections computed in transposed [feature, token] layout so no
    on-device transposes are needed anywhere.
  - rotate_half is pre-baked on the host into 64 extra "swapped" weight
    columns per q/k head, so rotary is 3 aligned elementwise ops.
  - V is projected through this head's slice of Wproj *before* the
    attention-weighted sum (U_h = Wproj_h @ v_h), shrinking the big
    j-contraction from d=256 to e=128 and making the output projection free.
  - Softmax without max-subtraction (|score*scale| < ~1 by construction);
    denominators via ones-vector matmuls; normalization + bias + head/core
    reduction on the host (cheap: 8MB of partial outputs).
  - bf16 matmul inputs, fp32 PSUM accumulation.
"""

import sys

sys.path.insert(0, "/opt/trn_rl_repo")

import numpy as np
import ml_dtypes

import concourse.bacc as bacc
import concourse.tile as tile
from concourse import mybir
from concourse.bass_utils import run_bass_kernel_spmd

DIM = 128
HEADS = 32
DH = 256          # per-head dim
ROT = 64          # partial rotary width
N = 2048
NCORES = 8
HPC = HEADS // NCORES  # heads per core = 4
SCALE = float(DIM) ** -0.5

BF16 = mybir.dt.bfloat16
FP8 = mybir.dt.float8e4
F32 = mybir.dt.float32
EXP = mybir.ActivationFunctionType.Exp
DR = mybir.MatmulPerfMode.DoubleRow

BF16_NP = ml_dtypes.bfloat16
FP8_NP = ml_dtypes.float8_e4m3


def build_nc(n=N):
    """Build the per-core Bass program (identical for all cores; SPMD)."""
    assert n % 512 == 0
    nch = n // 512   # 512-wide query chunks
    njt = n // 128   # 128-wide key tiles

    nc = bacc.Bacc("TRN2", target_bir_lowering=False, debug=False,
                   num_devices=NCORES)

    xT = nc.dram_tensor("xT", [128, n], BF16, kind="ExternalInput")
    wq = nc.dram_tensor("wq", [128, HPC, DH + ROT], BF16, kind="ExternalInput")
    wk = nc.dram_tensor("wk", [128, HPC, DH + ROT], BF16, kind="ExternalInput")
    wv = nc.dram_tensor("wv", [128, HPC, DH], BF16, kind="ExternalInput")
    wp = nc.dram_tensor("wp", [128, HPC, 2, 128], BF16, kind="ExternalInput")
    cosT = nc.dram_tensor("cosT", [ROT, n], F32, kind="ExternalInput")
    sinT = nc.dram_tensor("sinT", [ROT, n], F32, kind="ExternalInput")
    uv = nc.dram_tensor("uv", [HPC, 128, n], F32, kind="ExternalOutput")
    ssum = nc.dram_tensor("ssum", [HPC, 128, n], F32, kind="ExternalOutput")

    with tile.TileContext(nc) as tc:
        with (
            tc.tile_pool(name="consts", bufs=1) as consts,
            tc.tile_pool(name="hd", bufs=2) as hd,
            tc.tile_pool(name="es", bufs=4) as es,
            tc.tile_pool(name="tmp", bufs=4) as tmp,
            tc.tile_pool(name="ps", bufs=2, space="PSUM") as ps,
        ):
            wq_sb = consts.tile([128, HPC, DH + ROT], BF16)
            nc.sync.dma_start(out=wq_sb[:, 0, :], in_=wq[:, 0, :])
            xT_sb = consts.tile([128, n], BF16)
            for ci in range(nch):   # chunked so the first matmul starts early
                sl = slice(ci * 512, ci * 512 + 512)
                nc.sync.dma_start(out=xT_sb[:, sl], in_=xT[:, sl])
            for hh in range(1, HPC):
                nc.sync.dma_start(out=wq_sb[:, hh, :], in_=wq[:, hh, :])
            wk_sb = consts.tile([128, HPC, DH + ROT], BF16)
            nc.gpsimd.dma_start(out=wk_sb, in_=wk[:, :, :])
            wv_sb = consts.tile([128, HPC, DH], BF16)
            nc.gpsimd.dma_start(out=wv_sb, in_=wv[:, :, :])
            wp_sb = consts.tile([128, HPC, 2, 128], BF16)
            nc.gpsimd.dma_start(out=wp_sb, in_=wp[:, :, :, :])
            cos_sb = consts.tile([ROT, n], F32)
            nc.gpsimd.dma_start(out=cos_sb, in_=cosT[:, :])
            sin_sb = consts.tile([ROT, n], F32)
            nc.gpsimd.dma_start(out=sin_sb, in_=sinT[:, :])

            def alloc_head_tiles():
                qT_sb = hd.tile([128, 2, n], FP8, tag="q", name="qT_sb")
                kT_sb = hd.tile([128, 2, n], FP8, tag="k", name="kT_sb")
                vT_sb = hd.tile([128, 2, n], BF16, tag="v", name="vT_sb")
                U_sb = hd.tile([128, njt, 128], BF16, tag="u", name="U_sb")
                return qT_sb, kT_sb, vT_sb, U_sb

            def emit_qk(h, w_sb, outT):
                # q or k projection with fused rotary
                for dt in range(2):
                    dsl = slice(dt * 128, dt * 128 + 128)
                    for ci in range(nch):
                        sl = slice(ci * 512, ci * 512 + 512)
                        psq = ps.tile([128, 512], F32, tag="pqk", bufs=3,
                                      name="psq")
                        nc.tensor.matmul(psq, w_sb[:, h, dsl],
                                         xT_sb[:, sl], start=True, stop=True)
                        if dt == 0:
                            # swapped-partner projection for rotate_half
                            psqs = ps.tile([64, 512], F32, tag="pqk", bufs=3,
                                           name="psqs")
                            nc.tensor.matmul(psqs, w_sb[:, h, DH:DH + ROT],
                                             xT_sb[:, sl], start=True, stop=True)
                            t1 = tmp.tile([64, 512], F32, tag="t1", name="t1")
                            t2 = tmp.tile([64, 512], F32, tag="t2", name="t2")
                            nc.vector.tensor_mul(t1, psq[0:ROT, :], cos_sb[:, sl])
                            nc.vector.tensor_mul(t2, psqs[:, :], sin_sb[:, sl])
                            nc.vector.tensor_add(outT[0:ROT, dt, sl], t1, t2)
                            nc.scalar.copy(outT[ROT:128, dt, sl], psq[ROT:128, :])
                        else:
                            nc.scalar.copy(outT[:, dt, sl], psq)

            def emit_v(h, vT_sb):
                for dt in range(2):
                    dsl = slice(dt * 128, dt * 128 + 128)
                    for ci in range(nch):
                        sl = slice(ci * 512, ci * 512 + 512)
                        psv = ps.tile([128, 512], F32, tag="pqk", bufs=3,
                                      name="psv")
                        nc.tensor.matmul(psv, wv_sb[:, h, dsl], xT_sb[:, sl],
                                         start=True, stop=True)
                        nc.scalar.copy(vT_sb[:, dt, sl], psv)

            def emit_u(h, vT_sb, U_sb):
                # U_h[j, e] = sum_d vT[d, j] * WprojT[(h,d), e]
                for jt in range(njt):
                    jsl = slice(jt * 128, jt * 128 + 128)
                    psu = ps.tile([128, 128], F32, tag="pqk", bufs=3, name="psu")
                    for dt in range(2):
                        nc.tensor.matmul(psu, vT_sb[:, dt, jsl], wp_sb[:, h, dt, :],
                                         start=(dt == 0), stop=(dt == 1))
                    nc.vector.tensor_copy(U_sb[:, jt, :], psu)

            def build_parts(h, tiles):
                qT_sb, kT_sb, vT_sb, U_sb = tiles
                return [
                    lambda: emit_qk(h, wq_sb, qT_sb),
                    lambda: emit_qk(h, wk_sb, kT_sb),
                    lambda: emit_v(h, vT_sb),
                    lambda: emit_u(h, vT_sb, U_sb),
                ]

            cur = alloc_head_tiles()
            for part in build_parts(0, cur):
                part()

            for h in range(HPC):
                qT_sb, kT_sb, vT_sb, U_sb = cur
                if h + 1 < HPC:
                    nxt = alloc_head_tiles()
                    nxt_parts = build_parts(h + 1, nxt)
                else:
                    nxt, nxt_parts = None, None

                # ---- attention: fp8-DR scores -> paired exp -> bf16 U-sum ----
                # next head's build work is interleaved between query chunks
                # so the PE fills attention stalls and DVE/ACT load spreads.
                for ci in range(nch):
                    isl = slice(ci * 512, ci * 512 + 512)
                    expS = es.tile([128, njt, 512], BF16, tag="e")
                    psuv = ps.tile([128, 512], F32, tag="puv", bufs=1)
                    for jp in range(njt // 2):
                        pss = ps.tile([128, 1024], F32, tag="ps", bufs=2)
                        for u in range(2):
                            jt = 2 * jp + u
                            jsl = slice(jt * 128, jt * 128 + 128)
                            nc.tensor.matmul(pss[:, u * 512:(u + 1) * 512],
                                             kT_sb[:, :, jsl], qT_sb[:, :, isl],
                                             start=True, stop=True, perf_mode=DR)
                        nc.scalar.activation(
                            expS[:, 2 * jp:2 * jp + 2, :].rearrange("p j i -> p (j i)"),
                            pss, EXP, scale=SCALE)
                        for u in range(2):
                            jt = 2 * jp + u
                            nc.tensor.matmul(psuv, U_sb[:, jt, :], expS[:, jt, :],
                                             start=(jt == 0), stop=(jt == njt - 1))
                    # evict the chunk output first so the single psuv
                    # bank frees before the long denominator chain
                    ouv = tmp.tile([128, 512], F32, tag="ouv", bufs=2)
                    nc.vector.tensor_copy(ouv, psuv)
                    nc.sync.dma_start(out=uv[h, :, isl], in_=ouv)
                    # denominator partials (sum over key tiles), off the PE:
                    # DVE reduces 6 tiles, GpSimd add-tree the other 10.
                    nd = max(2, njt * 6 // 16)
                    sdve = tmp.tile([128, 512], F32, tag="sdve", bufs=2)
                    nc.vector.tensor_reduce(
                        sdve, expS[:, 0:nd, :].rearrange("p j i -> p i j"),
                        axis=mybir.AxisListType.X, op=mybir.AluOpType.add)
                    g = [expS[:, nd + t, :] for t in range(njt - nd)]
                    while len(g) > 2:
                        lvl = []
                        for a, b in zip(g[0::2], g[1::2]):
                            gt = tmp.tile([128, 512], BF16, tag="gtree", bufs=8)
                            nc.gpsimd.tensor_add(gt, a, b)
                            lvl.append(gt)
                        if len(g) % 2:
                            lvl.append(g[-1])
                        g = lvl
                    gsum = tmp.tile([128, 512], F32, tag="gsum", bufs=2)
                    nc.gpsimd.tensor_add(gsum, g[0], g[1])
                    osm = tmp.tile([128, 512], F32, tag="osm", bufs=2)
                    nc.vector.tensor_add(osm, sdve, gsum)
                    nc.sync.dma_start(out=ssum[h, :, isl], in_=osm)
                    if nxt_parts is not None and ci < len(nxt_parts):
                        nxt_parts[ci]()
                if nxt_parts is not None:
                    for pi in range(nch, len(nxt_parts)):
                        nxt_parts[pi]()
                cur = nxt

    nc.compile()
    return nc


_PERM = np.concatenate([np.arange(32, 64), np.arange(0, 32)])


def prep_core(core, x, Wqkv, Wproj, rot, n=N):
    """Build the per-core input map (numpy, host-side sharding/layout)."""
    hs = slice(core * HPC, (core + 1) * HPC)
    W4 = Wqkv.reshape(3, HEADS, DH, DIM)

    def qk_aug(w):  # w: [HPC, DH, DIM] -> [128, HPC, DH+ROT]
        wt = np.ascontiguousarray(w.transpose(2, 0, 1))          # [128, HPC, 256]
        sw = np.ascontiguousarray(w[:, _PERM, :].transpose(2, 0, 1))  # [128, HPC, 64]
        return np.concatenate([wt, sw], axis=2).astype(BF16_NP)

    wq_h = qk_aug(W4[0, hs])
    wk_h = qk_aug(W4[1, hs])
    wv_h = np.ascontiguousarray(W4[2, hs].transpose(2, 0, 1)).astype(BF16_NP)
    # Wproj [128 e, INNER] -> this core's [(h,d)] rows as [128 d_in, HPC, 2, 128 e]
    Wp4 = Wproj.reshape(DIM, HEADS, DH)[:, hs, :]                # [128 e, HPC, 256]
    wp_h = np.ascontiguousarray(
        Wp4.reshape(DIM, HPC, 2, 128).transpose(3, 1, 2, 0)).astype(BF16_NP)

    cosT = np.ascontiguousarray(np.cos(rot).T).astype(np.float32)   # [64, n]
    sinT = np.ascontiguousarray(np.sin(rot).T).astype(np.float32)
    sinT[:32] *= -1.0   # sign of rotate_half for output rows 0:32

    xT = np.ascontiguousarray(x.reshape(n, DIM).T).astype(BF16_NP)  # [128, n]

    return {
        "xT": xT, "wq": wq_h, "wk": wk_h, "wv": wv_h, "wp": wp_h,
        "cosT": cosT, "sinT": sinT,
    }


def postprocess(results, bproj, n=N):
    """Normalize per (head, query), sum heads/cores, add bias -> [1, n, 128]."""
    acc = np.zeros((DIM, n), np.float64)
    for r in results:
        uv = np.asarray(r["uv"], np.float64)       # [HPC, 128, n]
        ssum = np.asarray(r["ssum"], np.float64)   # [HPC, 128, n] partials
        ssum = ssum.sum(axis=1)                    # [HPC, n]
        acc += (uv / ssum[:, None, :]).sum(axis=0)
    out = acc.T + np.asarray(bproj, np.float64)[None, :]
    return out.astype(np.float32).reshape(1, n, DIM)


_NC_CACHE = {}


def _get_nc(n=N):
    if n not in _NC_CACHE:
        _NC_CACHE[n] = build_nc(n)
    return _NC_CACHE[n]


def kernel(x, Wqkv, Wproj, bproj, rotary_pos_emb):
    x = np.asarray(x, np.float32)
    Wqkv = np.asarray(Wqkv, np.float32)
    Wproj = np.asarray(Wproj, np.float32)
    bproj = np.asarray(bproj, np.float32)
    rot = np.asarray(rotary_pos_emb, np.float32)

    nc = _get_nc(N)
    in_maps = [prep_core(c, x, Wqkv, Wproj, rot, N) for c in range(NCORES)]
    res = run_bass_kernel_spmd(nc, in_maps, core_ids=list(range(NCORES)))
    return postprocess(res.results, bproj, N)


# revision 25
# speedup vs baseline: 1.1244x; 1.0450x over previous
"""Trainium2 Bass kernel for nn_Attention (32-head attention, partial rotary,
dense softmax) sharded 4-heads-per-core across 8 NeuronCores.

Self-contained: takes full unsharded inputs, returns the full output.

Design notes (per core, heads h = 4c..4c+3, N=2048 tokens, d_head=256, e=128):
  - All projections computed in transposed [feature, token] layout so no
    on-device transposes are needed anywhere.
  - rotate_half is pre-baked on the host into 64 extra "swapped" weight
    columns per q/k head, so rotary is 3 aligned elementwise ops.
  - V is projected through this head's slice of Wproj *before* the
    attention-weighted sum (U_h = Wproj_h @ v_h), shrinking the big
    j-contraction from d=256 to e=128 and making the output projection free.
  - Softmax without max-subtraction (|score*scale| < ~1 by construction);
    denominators via ones-vector matmuls; normalization + bias + head/core
    reduction on the host (cheap: 8MB of partial outputs).
  - bf16 matmul inputs, fp32 PSUM accumulation.
"""

import sys

sys.path.insert(0, "/opt/trn_rl_repo")

import numpy as np
import ml_dtypes

import concourse.bacc as bacc
import concourse.tile as tile
from concourse import mybir
from concourse.bass_utils import run_bass_kernel_spmd

DIM = 128
HEADS = 32
DH = 256          # per-head dim
ROT = 64          # partial rotary width
N = 2048
NCORES = 8
HPC = HEADS // NCORES  # heads per core = 4
SCALE = float(DIM) ** -0.5

BF16 = mybir.dt.bfloat16
FP8 = mybir.dt.float8e4
F32 = mybir.dt.float32
EXP = mybir.ActivationFunctionType.Exp
DR = mybir.MatmulPerfMode.DoubleRow

BF16_NP = ml_dtypes.bfloat16
FP8_NP = ml_dtypes.float8_e4m3


def build_nc(n=N):
    """Build the per-core Bass program (identical for all cores; SPMD)."""
    assert n % 512 == 0
    nch = n // 512   # 512-wide query chunks
    njt = n // 128   # 128-wide key tiles

    nc = bacc.Bacc("TRN2", target_bir_lowering=False, debug=False,
                   num_devices=NCORES)

    xT = nc.dram_tensor("xT", [128, n], BF16, kind="ExternalInput")
    wq = nc.dram_tensor("wq", [128, HPC, DH + ROT], BF16, kind="ExternalInput")
    wk = nc.dram_tensor("wk", [128, HPC, DH + ROT], BF16, kind="ExternalInput")
    wv = nc.dram_tensor("wv", [128, HPC, DH], BF16, kind="ExternalInput")
    wp = nc.dram_tensor("wp", [128, HPC, 2, 128], BF16, kind="ExternalInput")
    cosT = nc.dram_tensor("cosT", [ROT, n], F32, kind="ExternalInput")
    sinT = nc.dram_tensor("sinT", [ROT, n], F32, kind="ExternalInput")
    uv = nc.dram_tensor("uv", [HPC, 128, n], F32, kind="ExternalOutput")
    ssum = nc.dram_tensor("ssum", [HPC, 128, n], F32, kind="ExternalOutput")
    # final chunk's exp tiles raw; host does its denominator sum (cuts tail)
    etail = nc.dram_tensor("etail", [128, n // 128, 512], BF16,
                           kind="ExternalOutput")

    with tile.TileContext(nc) as tc:
        with (
            tc.tile_pool(name="consts", bufs=1) as consts,
            tc.tile_pool(name="hd", bufs=2) as hd,
            tc.tile_pool(name="es", bufs=4) as es,
            tc.tile_pool(name="tmp", bufs=4) as tmp,
            tc.tile_pool(name="ps", bufs=2, space="PSUM") as ps,
        ):
            wq_sb = consts.tile([128, HPC, DH + ROT], BF16)
            nc.sync.dma_start(out=wq_sb[:, 0, :], in_=wq[:, 0, :])
            xT_sb = consts.tile([128, n], BF16)
            for ci in range(nch):   # chunked so the first matmul starts early
                sl = slice(ci * 512, ci * 512 + 512)
                nc.sync.dma_start(out=xT_sb[:, sl], in_=xT[:, sl])
            for hh in range(1, HPC):
                nc.sync.dma_start(out=wq_sb[:, hh, :], in_=wq[:, hh, :])
            wk_sb = consts.tile([128, HPC, DH + ROT], BF16)
            nc.gpsimd.dma_start(out=wk_sb, in_=wk[:, :, :])
            wv_sb = consts.tile([128, HPC, DH], BF16)
            nc.gpsimd.dma_start(out=wv_sb, in_=wv[:, :, :])
            wp_sb = consts.tile([128, HPC, 2, 128], BF16)
            nc.gpsimd.dma_start(out=wp_sb, in_=wp[:, :, :, :])
            cos_sb = consts.tile([ROT, n], F32)
            nc.gpsimd.dma_start(out=cos_sb, in_=cosT[:, :])
            sin_sb = consts.tile([ROT, n], F32)
            nc.gpsimd.dma_start(out=sin_sb, in_=sinT[:, :])

            def alloc_head_tiles():
                qT_sb = hd.tile([128, 2, n], FP8, tag="q", name="qT_sb")
                kT_sb = hd.tile([128, 2, n], FP8, tag="k", name="kT_sb")
                vT_sb = hd.tile([128, 2, n], BF16, tag="v", name="vT_sb")
                U_sb = hd.tile([128, njt, 128], BF16, tag="u", name="U_sb")
                return qT_sb, kT_sb, vT_sb, U_sb

            def emit_qk(h, w_sb, outT):
                # q or k projection with fused rotary
                for dt in range(2):
                    dsl = slice(dt * 128, dt * 128 + 128)
                    for ci in range(nch):
                        sl = slice(ci * 512, ci * 512 + 512)
                        psq = ps.tile([128, 512], F32, tag="pqk", bufs=3,
                                      name="psq")
                        nc.tensor.matmul(psq, w_sb[:, h, dsl],
                                         xT_sb[:, sl], start=True, stop=True)
                        if dt == 0:
                            # swapped-partner projection for rotate_half
                            psqs = ps.tile([64, 512], F32, tag="pqk", bufs=3,
                                           name="psqs")
                            nc.tensor.matmul(psqs, w_sb[:, h, DH:DH + ROT],
                                             xT_sb[:, sl], start=True, stop=True)
                            t1 = tmp.tile([64, 512], F32, tag="t1", name="t1")
                            t2 = tmp.tile([64, 512], F32, tag="t2", name="t2")
                            nc.vector.tensor_mul(t1, psq[0:ROT, :], cos_sb[:, sl])
                            nc.vector.tensor_mul(t2, psqs[:, :], sin_sb[:, sl])
                            nc.vector.tensor_add(outT[0:ROT, dt, sl], t1, t2)
                            nc.scalar.copy(outT[ROT:128, dt, sl], psq[ROT:128, :])
                        else:
                            nc.scalar.copy(outT[:, dt, sl], psq)

            def emit_v(h, vT_sb):
                for dt in range(2):
                    dsl = slice(dt * 128, dt * 128 + 128)
                    for ci in range(nch):
                        sl = slice(ci * 512, ci * 512 + 512)
                        psv = ps.tile([128, 512], F32, tag="pqk", bufs=3,
                                      name="psv")
                        nc.tensor.matmul(psv, wv_sb[:, h, dsl], xT_sb[:, sl],
                                         start=True, stop=True)
                        nc.scalar.copy(vT_sb[:, dt, sl], psv)

            def emit_u(h, vT_sb, U_sb):
                # U_h[j, e] = sum_d vT[d, j] * WprojT[(h,d), e]
                for jt in range(njt):
                    jsl = slice(jt * 128, jt * 128 + 128)
                    psu = ps.tile([128, 128], F32, tag="pqk", bufs=3, name="psu")
                    for dt in range(2):
                        nc.tensor.matmul(psu, vT_sb[:, dt, jsl], wp_sb[:, h, dt, :],
                                         start=(dt == 0), stop=(dt == 1))
                    nc.vector.tensor_copy(U_sb[:, jt, :], psu)

            def build_parts(h, tiles):
                qT_sb, kT_sb, vT_sb, U_sb = tiles
                return [
                    lambda: emit_qk(h, wq_sb, qT_sb),
                    lambda: emit_qk(h, wk_sb, kT_sb),
                    lambda: emit_v(h, vT_sb),
                    lambda: emit_u(h, vT_sb, U_sb),
                ]

            cur = alloc_head_tiles()
            for part in build_parts(0, cur):
                part()

            for h in range(HPC):
                qT_sb, kT_sb, vT_sb, U_sb = cur
                if h + 1 < HPC:
                    nxt = alloc_head_tiles()
                    nxt_parts = build_parts(h + 1, nxt)
                else:
                    nxt, nxt_parts = None, None

                # ---- attention: fp8-DR scores -> paired exp -> bf16 U-sum ----
                # next head's build work is interleaved between query chunks
                # so the PE fills attention stalls and DVE/ACT load spreads.
                for ci in range(nch):
                    isl = slice(ci * 512, ci * 512 + 512)
                    expS = es.tile([128, njt, 512], BF16, tag="e")
                    psuv = ps.tile([128, 512], F32, tag="puv", bufs=1)
                    for jp in range(njt // 2):
                        pss = ps.tile([128, 1024], F32, tag="ps", bufs=2)
                        for u in range(2):
                            jt = 2 * jp + u
                            jsl = slice(jt * 128, jt * 128 + 128)
                            nc.tensor.matmul(pss[:, u * 512:(u + 1) * 512],
                                             kT_sb[:, :, jsl], qT_sb[:, :, isl],
                                             start=True, stop=True, perf_mode=DR)
                        nc.scalar.activation(
                            expS[:, 2 * jp:2 * jp + 2, :].rearrange("p j i -> p (j i)"),
                            pss, EXP, scale=SCALE)
                        for u in range(2):
                            jt = 2 * jp + u
                            nc.tensor.matmul(psuv, U_sb[:, jt, :], expS[:, jt, :],
                                             start=(jt == 0), stop=(jt == njt - 1))
                    # evict the chunk output first so the single psuv
                    # bank frees before the long denominator chain
                    ouv = tmp.tile([128, 512], F32, tag="ouv", bufs=2)
                    nc.vector.tensor_copy(ouv, psuv)
                    nc.sync.dma_start(out=uv[h, :, isl], in_=ouv)
                    # next head's build part goes ahead of the denominator
                    # chain so its rotary/evicts aren't stuck behind the
                    # long DVE reduce in the queue
                    if nxt_parts is not None and ci < len(nxt_parts):
                        nxt_parts[ci]()
                    if h == HPC - 1 and ci == nch - 1:
                        # last chunk: ship raw exp tiles; host sums them
                        for jt in range(njt):
                            nc.sync.dma_start(out=etail[:, jt, :],
                                              in_=expS[:, jt, :])
                        continue
                    # denominator partials (sum over key tiles), off the PE:
                    # DVE reduces 6 tiles, GpSimd add-tree the other 10.
                    nd = max(2, njt * 6 // 16)
                    sdve = tmp.tile([128, 512], F32, tag="sdve", bufs=2)
                    nc.vector.tensor_reduce(
                        sdve, expS[:, 0:nd, :].rearrange("p j i -> p i j"),
                        axis=mybir.AxisListType.X, op=mybir.AluOpType.add)
                    g = [expS[:, nd + t, :] for t in range(njt - nd)]
                    while len(g) > 2:
                        lvl = []
                        for a, b in zip(g[0::2], g[1::2]):
                            gt = tmp.tile([128, 512], BF16, tag="gtree", bufs=8)
                            nc.gpsimd.tensor_add(gt, a, b)
                            lvl.append(gt)
                        if len(g) % 2:
                            lvl.append(g[-1])
                        g = lvl
                    gsum = tmp.tile([128, 512], F32, tag="gsum", bufs=2)
                    nc.gpsimd.tensor_add(gsum, g[0], g[1])
                    osm = tmp.tile([128, 512], F32, tag="osm", bufs=2)
                    nc.vector.tensor_add(osm, sdve, gsum)
                    nc.sync.dma_start(out=ssum[h, :, isl], in_=osm)
                if nxt_parts is not None:
                    for pi in range(nch, len(nxt_parts)):
                        nxt_parts[pi]()
                cur = nxt

    nc.compile()
    return nc


_PERM = np.concatenate([np.arange(32, 64), np.arange(0, 32)])


def prep_core(core, x, Wqkv, Wproj, rot, n=N):
    """Build the per-core input map (numpy, host-side sharding/layout)."""
    hs = slice(core * HPC, (core + 1) * HPC)
    W4 = Wqkv.reshape(3, HEADS, DH, DIM)

    def qk_aug(w):  # w: [HPC, DH, DIM] -> [128, HPC, DH+ROT]
        wt = np.ascontiguousarray(w.transpose(2, 0, 1))          # [128, HPC, 256]
        sw = np.ascontiguousarray(w[:, _PERM, :].transpose(2, 0, 1))  # [128, HPC, 64]
        return np.concatenate([wt, sw], axis=2).astype(BF16_NP)

    wq_h = qk_aug(W4[0, hs])
    wk_h = qk_aug(W4[1, hs])
    wv_h = np.ascontiguousarray(W4[2, hs].transpose(2, 0, 1)).astype(BF16_NP)
    # Wproj [128 e, INNER] -> this core's [(h,d)] rows as [128 d_in, HPC, 2, 128 e]
    Wp4 = Wproj.reshape(DIM, HEADS, DH)[:, hs, :]                # [128 e, HPC, 256]
    wp_h = np.ascontiguousarray(
        Wp4.reshape(DIM, HPC, 2, 128).transpose(3, 1, 2, 0)).astype(BF16_NP)

    cosT = np.ascontiguousarray(np.cos(rot).T).astype(np.float32)   # [64, n]
    sinT = np.ascontiguousarray(np.sin(rot).T).astype(np.float32)
    sinT[:32] *= -1.0   # sign of rotate_half for output rows 0:32

    xT = np.ascontiguousarray(x.reshape(n, DIM).T).astype(BF16_NP)  # [128, n]

    return {
        "xT": xT, "wq": wq_h, "wk": wk_h, "wv": wv_h, "wp": wp_h,
        "cosT": cosT, "sinT": sinT,
    }


def postprocess(results, bproj, n=N):
    """Normalize per (head, query), sum heads/cores, add bias -> [1, n, 128]."""
    acc = np.zeros((DIM, n), np.float64)
    for r in results:
        uv = np.asarray(r["uv"], np.float64)       # [HPC, 128, n]
        ssum = np.asarray(r["ssum"], np.float64)   # [HPC, 128, n] partials
        et = np.asarray(r["etail"], np.float64)    # [128, n//128, 512]
        ssum[-1, :, n - 512:] = et.sum(axis=1)     # last chunk summed on host
        ssum = ssum.sum(axis=1)                    # [HPC, n]
        acc += (uv / ssum[:, None, :]).sum(axis=0)
    out = acc.T + np.asarray(bproj, np.float64)[None, :]
    return out.astype(np.float32).reshape(1, n, DIM)


_NC_CACHE = {}


def _get_nc(n=N):
    if n not in _NC_CACHE:
        _NC_CACHE[n] = build_nc(n)
    return _NC_CACHE[n]


def kernel(x, Wqkv, Wproj, bproj, rotary_pos_emb):
    x = np.asarray(x, np.float32)
    Wqkv = np.asarray(Wqkv, np.float32)
    Wproj = np.asarray(Wproj, np.float32)
    bproj = np.asarray(bproj, np.float32)
    rot = np.asarray(rotary_pos_emb, np.float32)

    nc = _get_nc(N)
    in_maps = [prep_core(c, x, Wqkv, Wproj, rot, N) for c in range(NCORES)]
    res = run_bass_kernel_spmd(nc, in_maps, core_ids=list(range(NCORES)))
    return postprocess(res.results, bproj, N)


# revision 26
# speedup vs baseline: 1.1443x; 1.0177x over previous
"""Trainium2 Bass kernel for nn_Attention (32-head attention, partial rotary,
dense softmax) sharded 4-heads-per-core across 8 NeuronCores.

Self-contained: takes full unsharded inputs, returns the full output.

Design notes (per core, heads h = 4c..4c+3, N=2048 tokens, d_head=256, e=128):
  - All projections computed in transposed [feature, token] layout so no
    on-device transposes are needed anywhere.
  - rotate_half is pre-baked on the host into 64 extra "swapped" weight
    columns per q/k head, so rotary is 3 aligned elementwise ops.
  - V is projected through this head's slice of Wproj *before* the
    attention-weighted sum (U_h = Wproj_h @ v_h), shrinking the big
    j-contraction from d=256 to e=128 and making the output projection free.
  - Softmax without max-subtraction (|score*scale| < ~1 by construction);
    denominators via ones-vector matmuls; normalization + bias + head/core
    reduction on the host (cheap: 8MB of partial outputs).
  - bf16 matmul inputs, fp32 PSUM accumulation.
"""

import sys

sys.path.insert(0, "/opt/trn_rl_repo")

import numpy as np
import ml_dtypes

import concourse.bacc as bacc
import concourse.tile as tile
from concourse import mybir
from concourse.bass_utils import run_bass_kernel_spmd

DIM = 128
HEADS = 32
DH = 256          # per-head dim
ROT = 64          # partial rotary width
N = 2048
NCORES = 8
HPC = HEADS // NCORES  # heads per core = 4
SCALE = float(DIM) ** -0.5

BF16 = mybir.dt.bfloat16
FP8 = mybir.dt.float8e4
F32 = mybir.dt.float32
EXP = mybir.ActivationFunctionType.Exp
DR = mybir.MatmulPerfMode.DoubleRow

BF16_NP = ml_dtypes.bfloat16
FP8_NP = ml_dtypes.float8_e4m3


def build_nc(n=N):
    """Build the per-core Bass program (identical for all cores; SPMD)."""
    assert n % 512 == 0
    nch = n // 512   # 512-wide query chunks
    njt = n // 128   # 128-wide key tiles

    nc = bacc.Bacc("TRN2", target_bir_lowering=False, debug=False,
                   num_devices=NCORES)

    xT = nc.dram_tensor("xT", [128, n], BF16, kind="ExternalInput")
    wq = nc.dram_tensor("wq", [128, HPC, DH + ROT], BF16, kind="ExternalInput")
    wk = nc.dram_tensor("wk", [128, HPC, DH + ROT], BF16, kind="ExternalInput")
    wv = nc.dram_tensor("wv", [128, HPC, DH], BF16, kind="ExternalInput")
    wp = nc.dram_tensor("wp", [128, HPC, 2, 128], BF16, kind="ExternalInput")
    cosT = nc.dram_tensor("cosT", [ROT, n], F32, kind="ExternalInput")
    sinT = nc.dram_tensor("sinT", [ROT, n], F32, kind="ExternalInput")
    uv = nc.dram_tensor("uv", [HPC, 128, n], F32, kind="ExternalOutput")
    ssum = nc.dram_tensor("ssum", [HPC, 128, n], F32, kind="ExternalOutput")
    # final chunk's exp tiles raw; host does its denominator sum (cuts tail)
    etail = nc.dram_tensor("etail", [128, n // 128, 512], BF16,
                           kind="ExternalOutput")

    with tile.TileContext(nc) as tc:
        with (
            tc.tile_pool(name="consts", bufs=1) as consts,
            tc.tile_pool(name="hd", bufs=2) as hd,
            tc.tile_pool(name="es", bufs=4) as es,
            tc.tile_pool(name="tmp", bufs=4) as tmp,
            tc.tile_pool(name="ps", bufs=2, space="PSUM") as ps,
        ):
            wq_sb = consts.tile([128, HPC, DH + ROT], BF16)
            nc.sync.dma_start(out=wq_sb[:, 0, :], in_=wq[:, 0, :])
            xT_sb = consts.tile([128, n], BF16)
            for ci in range(nch):   # chunked so the first matmul starts early
                sl = slice(ci * 512, ci * 512 + 512)
                nc.sync.dma_start(out=xT_sb[:, sl], in_=xT[:, sl])
            for hh in range(1, HPC):
                nc.sync.dma_start(out=wq_sb[:, hh, :], in_=wq[:, hh, :])
            wk_sb = consts.tile([128, HPC, DH + ROT], BF16)
            nc.gpsimd.dma_start(out=wk_sb, in_=wk[:, :, :])
            wv_sb = consts.tile([128, HPC, DH], BF16)
            nc.gpsimd.dma_start(out=wv_sb, in_=wv[:, :, :])
            wp_sb = consts.tile([128, HPC, 2, 128], BF16)
            nc.gpsimd.dma_start(out=wp_sb, in_=wp[:, :, :, :])
            cos_sb = consts.tile([ROT, n], F32)
            nc.gpsimd.dma_start(out=cos_sb, in_=cosT[:, :])
            sin_sb = consts.tile([ROT, n], F32)
            nc.gpsimd.dma_start(out=sin_sb, in_=sinT[:, :])

            def alloc_head_tiles():
                qT_sb = hd.tile([128, 2, n], FP8, tag="q", name="qT_sb")
                kT_sb = hd.tile([128, 2, n], FP8, tag="k", name="kT_sb")
                vT_sb = hd.tile([128, 2, n], BF16, tag="v", name="vT_sb")
                U_sb = hd.tile([128, njt, 128], BF16, tag="u", name="U_sb")
                return qT_sb, kT_sb, vT_sb, U_sb

            def emit_qk(h, w_sb, outT):
                # q or k projection with fused rotary
                for dt in range(2):
                    dsl = slice(dt * 128, dt * 128 + 128)
                    for ci in range(nch):
                        sl = slice(ci * 512, ci * 512 + 512)
                        psq = ps.tile([128, 512], F32, tag="pqk", bufs=3,
                                      name="psq")
                        nc.tensor.matmul(psq, w_sb[:, h, dsl],
                                         xT_sb[:, sl], start=True, stop=True)
                        if dt == 0:
                            # swapped-partner projection for rotate_half
                            psqs = ps.tile([64, 512], F32, tag="pqk", bufs=3,
                                           name="psqs")
                            nc.tensor.matmul(psqs, w_sb[:, h, DH:DH + ROT],
                                             xT_sb[:, sl], start=True, stop=True)
                            t1 = tmp.tile([64, 512], F32, tag="t1", name="t1")
                            t2 = tmp.tile([64, 512], F32, tag="t2", name="t2")
                            nc.vector.tensor_mul(t1, psq[0:ROT, :], cos_sb[:, sl])
                            nc.vector.tensor_mul(t2, psqs[:, :], sin_sb[:, sl])
                            nc.vector.tensor_add(outT[0:ROT, dt, sl], t1, t2)
                            nc.scalar.copy(outT[ROT:128, dt, sl], psq[ROT:128, :])
                        else:
                            nc.scalar.copy(outT[:, dt, sl], psq)

            def emit_v(h, vT_sb):
                for dt in range(2):
                    dsl = slice(dt * 128, dt * 128 + 128)
                    for ci in range(nch):
                        sl = slice(ci * 512, ci * 512 + 512)
                        psv = ps.tile([128, 512], F32, tag="pqk", bufs=3,
                                      name="psv")
                        nc.tensor.matmul(psv, wv_sb[:, h, dsl], xT_sb[:, sl],
                                         start=True, stop=True)
                        nc.scalar.copy(vT_sb[:, dt, sl], psv)

            def emit_u(h, vT_sb, U_sb):
                # U_h[j, e] = sum_d vT[d, j] * WprojT[(h,d), e]
                # 4 key-tiles share one PSUM bank -> 1 eviction per 4
                for jt4 in range(0, njt, 4):
                    psu = ps.tile([128, 4, 128], F32, tag="pqk", bufs=3,
                                  name="psu")
                    for t in range(4):
                        jt = jt4 + t
                        jsl = slice(jt * 128, jt * 128 + 128)
                        for dt in range(2):
                            nc.tensor.matmul(psu[:, t, :], vT_sb[:, dt, jsl],
                                             wp_sb[:, h, dt, :],
                                             start=(dt == 0), stop=(dt == 1))
                    nc.vector.tensor_copy(U_sb[:, jt4:jt4 + 4, :], psu)

            def build_parts(h, tiles):
                qT_sb, kT_sb, vT_sb, U_sb = tiles
                return [
                    lambda: emit_qk(h, wq_sb, qT_sb),
                    lambda: emit_qk(h, wk_sb, kT_sb),
                    lambda: emit_v(h, vT_sb),
                    lambda: emit_u(h, vT_sb, U_sb),
                ]

            cur = alloc_head_tiles()
            for part in build_parts(0, cur):
                part()

            for h in range(HPC):
                qT_sb, kT_sb, vT_sb, U_sb = cur
                if h + 1 < HPC:
                    nxt = alloc_head_tiles()
                    nxt_parts = build_parts(h + 1, nxt)
                else:
                    nxt, nxt_parts = None, None

                # ---- attention: fp8-DR scores -> paired exp -> bf16 U-sum ----
                # next head's build work is interleaved between query chunks
                # so the PE fills attention stalls and DVE/ACT load spreads.
                for ci in range(nch):
                    isl = slice(ci * 512, ci * 512 + 512)
                    expS = es.tile([128, njt, 512], BF16, tag="e")
                    psuv = ps.tile([128, 512], F32, tag="puv", bufs=1)
                    for jp in range(njt // 2):
                        pss = ps.tile([128, 1024], F32, tag="ps", bufs=2)
                        for u in range(2):
                            jt = 2 * jp + u
                            jsl = slice(jt * 128, jt * 128 + 128)
                            nc.tensor.matmul(pss[:, u * 512:(u + 1) * 512],
                                             kT_sb[:, :, jsl], qT_sb[:, :, isl],
                                             start=True, stop=True, perf_mode=DR)
                        nc.scalar.activation(
                            expS[:, 2 * jp:2 * jp + 2, :].rearrange("p j i -> p (j i)"),
                            pss, EXP, scale=SCALE)
                        for u in range(2):
                            jt = 2 * jp + u
                            nc.tensor.matmul(psuv, U_sb[:, jt, :], expS[:, jt, :],
                                             start=(jt == 0), stop=(jt == njt - 1))
                    # evict the chunk output first so the single psuv
                    # bank frees before the long denominator chain
                    ouv = tmp.tile([128, 512], F32, tag="ouv", bufs=2)
                    nc.vector.tensor_copy(ouv, psuv)
                    nc.sync.dma_start(out=uv[h, :, isl], in_=ouv)
                    # next head's build part goes ahead of the denominator
                    # chain so its rotary/evicts aren't stuck behind the
                    # long DVE reduce in the queue
                    if nxt_parts is not None and ci < len(nxt_parts):
                        nxt_parts[ci]()
                    if h == HPC - 1 and ci == nch - 1:
                        # last chunk: ship raw exp tiles; host sums them
                        for jt in range(njt):
                            nc.sync.dma_start(out=etail[:, jt, :],
                                              in_=expS[:, jt, :])
                        continue
                    # denominator partials (sum over key tiles), off the PE:
                    # DVE reduces 6 tiles, GpSimd add-tree the other 10.
                    nd = max(2, njt * 6 // 16)
                    half_nd = nd // 2
                    sd1 = tmp.tile([128, 512], F32, tag="sd1", bufs=2)
                    nc.vector.tensor_reduce(
                        sd1, expS[:, 0:half_nd, :].rearrange("p j i -> p i j"),
                        axis=mybir.AxisListType.X, op=mybir.AluOpType.add)
                    sd2 = tmp.tile([128, 512], F32, tag="sd2", bufs=2)
                    nc.vector.tensor_reduce(
                        sd2, expS[:, half_nd:nd, :].rearrange("p j i -> p i j"),
                        axis=mybir.AxisListType.X, op=mybir.AluOpType.add)
                    sdve = tmp.tile([128, 512], F32, tag="sdve", bufs=2)
                    nc.vector.tensor_add(sdve, sd1, sd2)
                    g = [expS[:, nd + t, :] for t in range(njt - nd)]
                    while len(g) > 2:
                        lvl = []
                        for a, b in zip(g[0::2], g[1::2]):
                            gt = tmp.tile([128, 512], BF16, tag="gtree", bufs=8)
                            nc.gpsimd.tensor_add(gt, a, b)
                            lvl.append(gt)
                        if len(g) % 2:
                            lvl.append(g[-1])
                        g = lvl
                    gsum = tmp.tile([128, 512], F32, tag="gsum", bufs=2)
                    nc.gpsimd.tensor_add(gsum, g[0], g[1])
                    osm = tmp.tile([128, 512], F32, tag="osm", bufs=2)
                    nc.vector.tensor_add(osm, sdve, gsum)
                    nc.sync.dma_start(out=ssum[h, :, isl], in_=osm)
                if nxt_parts is not None:
                    for pi in range(nch, len(nxt_parts)):
                        nxt_parts[pi]()
                cur = nxt

    nc.compile()
    return nc


_PERM = np.concatenate([np.arange(32, 64), np.arange(0, 32)])


def prep_core(core, x, Wqkv, Wproj, rot, n=N):
    """Build the per-core input map (numpy, host-side sharding/layout)."""
    hs = slice(core * HPC, (core + 1) * HPC)
    W4 = Wqkv.reshape(3, HEADS, DH, DIM)

    def qk_aug(w):  # w: [HPC, DH, DIM] -> [128, HPC, DH+ROT]
        wt = np.ascontiguousarray(w.transpose(2, 0, 1))          # [128, HPC, 256]
        sw = np.ascontiguousarray(w[:, _PERM, :].transpose(2, 0, 1))  # [128, HPC, 64]
        return np.concatenate([wt, sw], axis=2).astype(BF16_NP)

    wq_h = qk_aug(W4[0, hs])
    wk_h = qk_aug(W4[1, hs])
    wv_h = np.ascontiguousarray(W4[2, hs].transpose(2, 0, 1)).astype(BF16_NP)
    # Wproj [128 e, INNER] -> this core's [(h,d)] rows as [128 d_in, HPC, 2, 128 e]
    Wp4 = Wproj.reshape(DIM, HEADS, DH)[:, hs, :]                # [128 e, HPC, 256]
    wp_h = np.ascontiguousarray(
        Wp4.reshape(DIM, HPC, 2, 128).transpose(3, 1, 2, 0)).astype(BF16_NP)

    cosT = np.ascontiguousarray(np.cos(rot).T).astype(np.float32)   # [64, n]
    sinT = np.ascontiguousarray(np.sin(rot).T).astype(np.float32)
    sinT[:32] *= -1.0   # sign of rotate_half for output rows 0:32

    xT = np.ascontiguousarray(x.reshape(n, DIM).T).astype(BF16_NP)  # [128, n]

    return {
        "xT": xT, "wq": wq_h, "wk": wk_h, "wv": wv_h, "wp": wp_h,
        "cosT": cosT, "sinT": sinT,
    }


def postprocess(results, bproj, n=N):
    """Normalize per (head, query), sum heads/cores, add bias -> [1, n, 128]."""
    acc = np.zeros((DIM, n), np.float64)
    for r in results:
        uv = np.asarray(r["uv"], np.float64)       # [HPC, 128, n]
        ssum = np.asarray(r["ssum"], np.float64)   # [HPC, 128, n] partials
        et = np.asarray(r["etail"], np.float64)    # [128, n//128, 512]
        ssum[-1, :, n - 512:] = et.sum(axis=1)     # last chunk summed on host
        ssum = ssum.sum(axis=1)                    # [HPC, n]
        acc += (uv / ssum[:, None, :]).sum(axis=0)
    out = acc.T + np.asarray(bproj, np.float64)[None, :]
    return out.astype(np.float32).reshape(1, n, DIM)


_NC_CACHE = {}


def _get_nc(n=N):
    if n not in _NC_CACHE:
        _NC_CACHE[n] = build_nc(n)
    return _NC_CACHE[n]


def kernel(x, Wqkv, Wproj, bproj, rotary_pos_emb):
    x = np.asarray(x, np.float32)
    Wqkv = np.asarray(Wqkv, np.float32)
    Wproj = np.asarray(Wproj, np.float32)
    bproj = np.asarray(bproj, np.float32)
    rot = np.asarray(rotary_pos_emb, np.float32)

    nc = _get_nc(N)
    in_maps = [prep_core(c, x, Wqkv, Wproj, rot, N) for c in range(NCORES)]
    res = run_bass_kernel_spmd(nc, in_maps, core_ids=list(range(NCORES)))
    return postprocess(res.results, bproj, N)
